# revision 19
# baseline (speedup 1.0000x reference)
"""DCGRU cell on 8 Trainium2 NeuronCores — data-parallel over batch.

Problem: nn_DCGRUCell (B=64, N=1024, D_IN=2, U=64, K=2, 2 supports).
Sharding: batch 64 -> 8 cores x 8 local batches (j). Supports + weights
replicated per core; everything else fully local, no collectives.

v4: transposed-domain formulation, instruction-count minimized.

The Chebyshev recursion is evaluated directly in the transposed
(feature-major) domain via matmuls against S^T and G = (S^T)^2
(precomputed on-chip once per input set):

    x1^T = x0^T @ S^T           (lhsT = x0 row-form, rhs = S^T)
    x~2^T = x0^T @ G            (x2 = 2*x~2 - x0 folded into weights)

so projection inputs come out of the PE already feature-major and no
layout transposes are needed for them; the only transpose per step is
r*hx back to row-form for the gconv2 diffusion lhsT (32 PE transposes).
The "2x - x0" recombination lives in the host-prepped weights
(slot-2/4 doubled, slot-0 absorbs -W2-W4), so diffusion psums evacuate
as plain copies.

Instruction-count measures (this execution path is per-instruction
dominated): 2-bank diffusion psums ([128,1024], one evac each), 4-bank
projection psums covering a j-pair ([128,2048], one act per 64-row
group), activations/blend on wide pair/quad tiles, merged DMAs.

Hardware rules honored (probed): matmuls of one accumulation group
share one operand start partition; tensor_tensor ops need all operands
at one start partition (start 0 here, odd-j data gets its own tiles);
engines may read PSUM across banks and at partition offsets.

Per-core layout (c = j*64 + u, c-tile t covers j in {2t, 2t+1}):
  hxr[nt]  [128, 512] bf16  row-form hx, lhsT for gconv1 diffusion
  hxt[t]   [128,1024] bf16  hx^T c-tiles (slot-0 rhs gconv1)
  hxtjO[t] [ 64,1024] bf16  hx^T odd j (start-0 vector operand)
  xt[mi][t][128,1024] bf16  diffusion slots (x1,x~2,x3,x~4)^T; doubles
                            as S row-form staging for the G build
  vr2/vu2  [ 64,8192] bf16  r|c and u, cols (t, p2, n)
  x0pT     [128,4096] bf16  (r*hx)^T c-tiles, cols (t, n); rows 0:64
                            by the even-j mul, 64:128 DMA-mirrored
  m1jO     [ 64,4096] bf16  (r*hx)^T odd j, cols (t, n)
  x0pr[nt] [128, 512] bf16  row-form r*hx via PE transposes
"""

import numpy as np
import ml_dtypes

import concourse.bass as bass
import concourse.tile as tile
import concourse.mybir as mybir
from concourse import bacc, masks
from concourse.bass_utils import run_bass_kernel_spmd

BF = mybir.dt.bfloat16
F32 = mybir.dt.float32
AF = mybir.ActivationFunctionType
OP = mybir.AluOpType

B, N, D_IN, U = 64, 1024, 2, 64
NCORES, J = 8, 8
NT = 8          # node tiles
CT = 4          # c tiles (j pairs)
O1, O2 = 2 * U, U

_CACHE = {}
STAGE = 99  # build cutoff for profiling components


def _build(reps=1):
    nc = bacc.Bacc(None)

    s0t_d = nc.dram_tensor("s0t", [N, N], BF, kind="ExternalInput")
    s1t_d = nc.dram_tensor("s1t", [N, N], BF, kind="ExternalInput")
    s0r_d = nc.dram_tensor("s0r", [N, N], BF, kind="ExternalInput")
    s1r_d = nc.dram_tensor("s1r", [N, N], BF, kind="ExternalInput")
    hxr_d = nc.dram_tensor("hxr", [N, J * U], BF, kind="ExternalInput")
    hxt_d = nc.dram_tensor("hxt", [J * U, N], BF, kind="ExternalInput")
    a0r_d = nc.dram_tensor("a0r", [N, 16], BF, kind="ExternalInput")
    a0t_d = nc.dram_tensor("a0t", [16, N], BF, kind="ExternalInput")
    wo_g_d = [nc.dram_tensor(f"wo_g{g}", [128, O1], BF, kind="ExternalInput") for g in range(5)]
    wu_g_d = [nc.dram_tensor(f"wu_g{g}", [128, O2], BF, kind="ExternalInput") for g in range(5)]
    wa_o_d = nc.dram_tensor("wa_o", [128, O1], BF, kind="ExternalInput")
    wa_u_d = nc.dram_tensor("wa_u", [128, O2], BF, kind="ExternalInput")
    b_or_d = nc.dram_tensor("b_or", [O2, 1], F32, kind="ExternalInput")
    b_ou_d = nc.dram_tensor("b_ou", [O2, 1], F32, kind="ExternalInput")
    b_uu_d = nc.dram_tensor("b_uu", [O2, 1], F32, kind="ExternalInput")
    out_d = nc.dram_tensor("out", [J * U, N], F32, kind="ExternalOutput")

    with tile.TileContext(nc) as tc:
        with (
            tc.tile_pool(name="const", bufs=1) as cp,
            tc.tile_pool(name="work", bufs=1) as wp,
            tc.tile_pool(name="bsp", bufs=2) as bsp,
            tc.tile_pool(name="s3p_", bufs=1) as s3pool,
            tc.tile_pool(name="pd", bufs=2, space="PSUM") as pdp,
            tc.tile_pool(name="pp", bufs=1, space="PSUM") as ppp,
        ):
            env = {}
            # ---- constants ----
            s0t = [cp.tile([128, N], BF, name=f"s0t{k}") for k in range(NT)]
            s1t = [cp.tile([128, N], BF, name=f"s1t{k}") for k in range(NT)]
            for k in range(NT):
                nc.sync.dma_start(s0t[k], s0t_d[k * 128:(k + 1) * 128, :])
                nc.sync.dma_start(s1t[k], s1t_d[k * 128:(k + 1) * 128, :])
            wo_g = [cp.tile([128, O1], BF, name=f"wo_g{g}") for g in range(5)]
            wu_g = [cp.tile([128, O2], BF, name=f"wu_g{g}") for g in range(5)]
            for g in range(5):
                nc.sync.dma_start(wo_g[g], wo_g_d[g][:, :])
                nc.sync.dma_start(wu_g[g], wu_g_d[g][:, :])
            wa_o = cp.tile([128, O1], BF, name="wa_o")
            wa_u = cp.tile([128, O2], BF, name="wa_u")
            nc.sync.dma_start(wa_o, wa_o_d[:, :])
            nc.sync.dma_start(wa_u, wa_u_d[:, :])
            b_or = cp.tile([O2, 1], F32, name="b_or")
            b_ou = cp.tile([O2, 1], F32, name="b_ou")
            b_uu = cp.tile([O2, 1], F32, name="b_uu")
            nc.sync.dma_start(b_or, b_or_d[:, :])
            nc.sync.dma_start(b_ou, b_ou_d[:, :])
            nc.sync.dma_start(b_uu, b_uu_d[:, :])
            ident = cp.tile([128, 128], BF, name="ident")
            masks.make_identity(nc, ident)

            hxr = [cp.tile([128, J * U], BF, name=f"hxr{k}") for k in range(NT)]
            hxt = [cp.tile([128, N], BF, name=f"hxt{t}") for t in range(CT)]
            hxtjO = [cp.tile([64, N], BF, name=f"hxtjO{t}") for t in range(CT)]
            for k in range(NT):
                nc.sync.dma_start(hxr[k], hxr_d[k * 128:(k + 1) * 128, :])
            for t in range(CT):
                nc.sync.dma_start(hxt[t], hxt_d[t * 128:(t + 1) * 128, :])
                nc.sync.dma_start(hxtjO[t], hxt_d[(2 * t + 1) * 64:(2 * t + 2) * 64, :])
            a0r = [cp.tile([128, 16], BF, name=f"a0r{k}") for k in range(NT)]
            a0t = cp.tile([16, N], BF, name="a0t")
            nc.sync.dma_start(a0t, a0t_d[:, :])
            for k in range(NT):
                nc.sync.dma_start(a0r[k], a0r_d[k * 128:(k + 1) * 128, :])

            # ---- persistent work tiles ----
            G0 = [wp.tile([128, N], BF, name=f"G0_{k}") for k in range(NT)]
            G1 = [wp.tile([128, N], BF, name=f"G1_{k}") for k in range(NT)]
            xt = [[wp.tile([128, N], BF, name=f"xt{mi}_{t}") for t in range(CT)]
                  for mi in range(4)]
            vr2 = wp.tile([64, 2 * CT * N], BF, name="vr2")
            vu2 = wp.tile([64, 2 * CT * N], BF, name="vu2")
            x0pT = wp.tile([128, CT * N], BF, name="x0pT")
            m1jO = wp.tile([64, CT * N], BF, name="m1jO")
            x0pr = [wp.tile([128, J * U], BF, name=f"x0pr{k}") for k in range(NT)]
            aT = [wp.tile([74, N], BF, name=f"aT{p}") for p in range(CT)]

            # ---- G = (S^T)^2, once per input set ----
            # S row-form staged in the xt tiles (free until the first
            # rep's diffusion overwrites them; Tile handles the WAR).
            sr0 = [xt[0][0], xt[0][1], xt[0][2], xt[0][3],
                   xt[1][0], xt[1][1], xt[1][2], xt[1][3]]
            sr1 = [xt[2][0], xt[2][1], xt[2][2], xt[2][3],
                   xt[3][0], xt[3][1], xt[3][2], xt[3][3]]
            for k in range(NT):
                nc.sync.dma_start(sr0[k], s0r_d[k * 128:(k + 1) * 128, :])
                nc.sync.dma_start(sr1[k], s1r_d[k * 128:(k + 1) * 128, :])
            for sr, stt_, G in ((sr0, s0t, G0), (sr1, s1t, G1)):
                for mt in range(NT):
                    pg = pdp.tile([128, N], F32, name=f"pg_{mt}", tag="pd")
                    for c2 in range(2):
                        cs = slice(c2 * 512, (c2 + 1) * 512)
                        for kt in range(NT):
                            nc.tensor.matmul(
                                pg[:, cs], sr[kt][:, mt * 128:(mt + 1) * 128],
                                stt_[kt][:, cs],
                                start=(kt == 0), stop=(kt == NT - 1),
                            )
                    nc.any.tensor_copy(G[mt], pg)

            # ---- A family: a_m^T [16, N], packed into aT pair tiles ----
            # stg staged in the low partitions of vr2 (free here).
            stg = [vr2[0:16, q * N:(q + 1) * N] for q in range(4)]
            for si, mtx in enumerate((s0t, G0, s1t, G1)):
                pa = ppp.tile([128, N], F32, name=f"pa_{si}", tag="pp")
                for c2 in range(2):
                    cs = slice(c2 * 512, (c2 + 1) * 512)
                    for kt in range(NT):
                        nc.tensor.matmul(
                            pa[0:16, cs], a0r[kt], mtx[kt][:, cs],
                            start=(kt == 0), stop=(kt == NT - 1),
                        )
                nc.any.tensor_copy(stg[si], pa[0:16, :])
            for j in range(J):
                jo = (j % 2) * 64
                nc.sync.dma_start(aT[j // 2][jo:jo + 2, :], a0t[2 * j:2 * j + 2, :])
                for m in range(4):
                    nc.sync.dma_start(aT[j // 2][jo + 2 * m + 2:jo + 2 * m + 4, :],
                                      stg[m][2 * j:2 * j + 2, :])

            env.update(locals())
            for rep in range(reps):
                _emit_body(env, rep)
    nc.compile()
    return nc


def _emit_body(env, rep):
    nc = env["nc"]
    s0t, s1t, G0, G1 = env["s0t"], env["s1t"], env["G0"], env["G1"]
    wo_g, wu_g = env["wo_g"], env["wu_g"]
    wa_o, wa_u = env["wa_o"], env["wa_u"]
    b_or, b_ou, b_uu = env["b_or"], env["b_ou"], env["b_uu"]
    hxr, hxt, hxtjO = env["hxr"], env["hxt"], env["hxtjO"]
    xt, aT = env["xt"], env["aT"]
    vr2, vu2 = env["vr2"], env["vu2"]
    x0pT, m1jO, x0pr = env["x0pT"], env["m1jO"], env["x0pr"]
    ident = env["ident"]
    pdp, ppp = env["pdp"], env["ppp"]
    bsp, s3pool = env["bsp"], env["s3pool"]
    out_d = env["out_d"]
    R = f"r{rep}"

    def diffuse(lhs, tag):
        # xt[mi]^T = x0^T @ M^T for M in (S0, S0^2, S1, S1^2); one
        # 2-bank psum + one evac per (matrix, c-tile).
        for mi, mtx in enumerate((s0t, G0, s1t, G1)):
            for mt in range(CT):
                pd = pdp.tile([128, N], F32, name=f"pd{R}{tag}_{mi}_{mt}", tag="pd")
                for c2 in range(2):
                    cs = slice(c2 * 512, (c2 + 1) * 512)
                    for kt in range(NT):
                        nc.tensor.matmul(
                            pd[:, cs], lhs[kt][:, mt * 128:(mt + 1) * 128],
                            mtx[kt][:, cs],
                            start=(kt == 0), stop=(kt == NT - 1),
                        )
                nc.any.tensor_copy(xt[mi][mt], pd)

    def slot0_g1(t):
        return hxt[t]

    def slot0_g2(t):
        return x0pT.rearrange("p (t n) -> p t n", t=CT)[:, t, :]

    def project(gc, slot0):
        # one 4-bank psum per (gconv, j-pair): cols (p2, c2); activations
        # read the whole 2048-wide row group at once.
        wg, wa = (wo_g, wa_o) if gc == 0 else (wu_g, wa_u)
        ob = O1 if gc == 0 else O2
        for t in range(CT):
            pp = ppp.tile([128, 2 * N], F32, name=f"pp{R}_{gc}_{t}", tag="pp")
            for p2 in range(2):
                jo = p2 * 64
                for c2 in range(2):
                    cs = slice(c2 * 512, (c2 + 1) * 512)
                    ps = pp[0:ob, p2 * N + c2 * 512:p2 * N + (c2 + 1) * 512]
                    nc.tensor.matmul(ps, wg[0][jo:jo + 64, :],
                                     slot0(t)[jo:jo + 64, cs],
                                     start=True, stop=False)
                    for i in range(1, 5):
                        nc.tensor.matmul(ps, wg[i][jo:jo + 64, :],
                                         xt[i - 1][t][jo:jo + 64, cs],
                                         start=False, stop=False)
                    nc.tensor.matmul(ps, wa[jo:jo + 10, :],
                                     aT[t][jo:jo + 10, cs],
                                     start=False, stop=True)
            tc_ = slice(t * 2 * N, (t + 1) * 2 * N)
            if gc == 0:
                nc.scalar.activation(out=vr2[:, tc_], in_=pp[0:64, :],
                                     func=AF.Sigmoid, bias=b_or, scale=1.0)
                nc.scalar.activation(out=vu2[:, tc_], in_=pp[64:128, :],
                                     func=AF.Sigmoid, bias=b_ou, scale=1.0)
            else:
                # tanh c overwrites the (dead) r columns
                nc.scalar.activation(out=vr2[:, tc_], in_=pp[0:64, :],
                                     func=AF.Tanh, bias=b_uu, scale=1.0)

    # ================= gconv 1 =================
    if STAGE < 1:
        return
    diffuse(hxr, "a")
    if STAGE < 2:
        return
    project(0, slot0_g1)
    if STAGE < 3:
        return

    # x0' = r*hx (even j straight into c-tile rows 0:64, odd j via one
    # DMA mirror), then PE transposes to row-form for the gconv2 lhsT.
    for t in range(CT):
        nc.vector.tensor_mul(x0pT[0:64, t * N:(t + 1) * N],
                             vr2[:, t * 2 * N:t * 2 * N + N], hxt[t][0:64, :])
        nc.vector.tensor_mul(m1jO[:, t * N:(t + 1) * N],
                             vr2[:, t * 2 * N + N:(t + 1) * 2 * N], hxtjO[t])
    nc.sync.dma_start(x0pT[64:128, :], m1jO)
    for nt_ in range(NT):
        pt = pdp.tile([128, 512], BF, name=f"pt{R}_{nt_}", tag="pd")
        for t in range(CT):
            nc.tensor.transpose(pt[:, t * 128:(t + 1) * 128],
                                x0pT[:, t * N + nt_ * 128:t * N + (nt_ + 1) * 128],
                                ident)
        nc.any.tensor_copy(x0pr[nt_], pt)
    if STAGE < 4:
        return

    # ================= gconv 2 =================
    diffuse(x0pr, "b")
    if STAGE < 5:
        return
    project(1, slot0_g2)
    if STAGE < 6:
        return

    # ---- blend in the transposed domain: out = c + u*(hx - c) ----
    for t in range(CT):
        tc_ = slice(t * 2 * N, (t + 1) * 2 * N)
        s1 = bsp.tile([64, 2 * N], BF, name=f"bs{R}_{t}a", tag="bs")
        s2 = bsp.tile([64, 2 * N], BF, name=f"bs{R}_{t}b", tag="bs")
        s3 = s3pool.tile([64, 2 * N], F32, name=f"s3{R}_{t}", tag="s3")
        nc.vector.scalar_tensor_tensor(
            out=s1[:, 0:N], in0=vr2[:, t * 2 * N:t * 2 * N + N], scalar=-1.0,
            in1=hxt[t][0:64, :], op0=OP.mult, op1=OP.add)
        nc.vector.scalar_tensor_tensor(
            out=s1[:, N:2 * N], in0=vr2[:, t * 2 * N + N:(t + 1) * 2 * N],
            scalar=-1.0, in1=hxtjO[t], op0=OP.mult, op1=OP.add)
        nc.vector.tensor_mul(s2, vu2[:, tc_], s1)
        nc.vector.tensor_add(s3, vr2[:, tc_], s2)
        if STAGE < 7:
            continue
        nc.sync.dma_start(
            out_d[t * 128:(t + 1) * 128, :].rearrange("(p u) n -> u p n", p=2),
            s3.rearrange("u (p n) -> u p n", p=2))


def _prep_shared(weights_output, biases_output, weights_update, biases_update):
    bf = ml_dtypes.bfloat16
    maps = {}
    for tag, W, ob in (("o", weights_output, O1), ("u", weights_update, O2)):
        Wr = W.reshape(66, 5, ob)
        H = Wr[2:, :, :]
        A = Wr[:2, :, :]
        # Chebyshev recombination x2 = 2*x~2 - x0 folded into the weights:
        # the kernel feeds raw x~2 = x0^T G, so slot-2/4 weights double and
        # slot-0 absorbs -W2-W4.
        Hm = [H[:, 0] - H[:, 2] - H[:, 4], H[:, 1], 2 * H[:, 2],
              H[:, 3], 2 * H[:, 4]]
        for i in range(5):
            blk = np.concatenate([Hm[i], Hm[i]])   # rows duplicated at 0/64
            maps[f"w{tag}_g{i}"] = np.ascontiguousarray(blk).astype(bf)
        At = A.transpose(1, 0, 2)                  # [5 m, 2 f, ob]
        Am = np.stack([At[0] - At[2] - At[4], At[1], 2 * At[2],
                       At[3], 2 * At[4]])
        wa = Am.reshape(10, ob)
        wa_pad = np.zeros((128, ob), np.float32)
        wa_pad[0:10] = wa
        wa_pad[64:74] = wa
        maps[f"wa_{tag}"] = wa_pad.astype(bf)
    bo = biases_output.astype(np.float32)
    maps["b_or"] = np.ascontiguousarray(bo[:U, None])
    maps["b_ou"] = np.ascontiguousarray(bo[U:, None])
    maps["b_uu"] = np.ascontiguousarray(biases_update.astype(np.float32)[:, None])
    return maps


def make_in_maps(inputs, hx, support0, support1, weights_output, biases_output,
                 weights_update, biases_update):
    bf = ml_dtypes.bfloat16
    shared = _prep_shared(np.asarray(weights_output, dtype=np.float32),
                          np.asarray(biases_output, dtype=np.float32),
                          np.asarray(weights_update, dtype=np.float32),
                          np.asarray(biases_update, dtype=np.float32))
    s0 = np.asarray(support0, np.float32)
    s1 = np.asarray(support1, np.float32)
    shared["s0t"] = np.ascontiguousarray(s0.T).astype(bf)
    shared["s1t"] = np.ascontiguousarray(s1.T).astype(bf)
    shared["s0r"] = np.ascontiguousarray(s0).astype(bf)
    shared["s1r"] = np.ascontiguousarray(s1).astype(bf)

    hx = np.asarray(hx, dtype=np.float32)
    xi = np.asarray(inputs, dtype=np.float32).reshape(B, N, D_IN)
    hx3 = hx.reshape(B, N, U)

    in_maps = []
    for c in range(NCORES):
        sl = slice(c * J, (c + 1) * J)
        hxc = hx3[sl].transpose(1, 0, 2).reshape(N, J * U)
        a0 = xi[sl].transpose(1, 0, 2).reshape(N, 16)   # [n, (j,f)]
        m = dict(shared)
        m["hxr"] = hxc.astype(bf)
        m["hxt"] = np.ascontiguousarray(hxc.T).astype(bf)
        m["a0r"] = a0.astype(bf)
        m["a0t"] = np.ascontiguousarray(a0.T).astype(bf)
        in_maps.append(m)
    return in_maps


def kernel(inputs, hx, support0, support1, weights_output, biases_output,
           weights_update, biases_update):
    if "nc" not in _CACHE:
        _CACHE["nc"] = _build()
    nc = _CACHE["nc"]
    in_maps = make_in_maps(inputs, hx, support0, support1, weights_output,
                           biases_output, weights_update, biases_update)
    res = run_bass_kernel_spmd(nc, in_maps, core_ids=list(range(NCORES)))
    outs = []
    for r in res.results:
        o = r["out"].reshape(J, U, N).transpose(0, 2, 1).reshape(J, N * U)
        outs.append(o)
    return np.concatenate(outs, axis=0)
